# revision 1
# baseline (speedup 1.0000x reference)
"""SimCLR contrastive loss on 8 TRN2 NeuronCores.

Strategy (per spec sharding_hint): shard the N=8192 anchors row-wise across
8 cores; replicate the normalized pred/positive matrices. Normalization and
transposition are cheap O(N*D) host work; the O(N^2) similarity + exp +
row-reduction runs on device and never materializes the NxN matrices.

Host side (in kernel()):
  - L2-normalize rows of pred/positive (torch-style eps clamp).
  - s[i] = zp_i . zq_i  (positive-pair logit, exact diag of the pq matrix).
  - Build zpT/zqT = normalized matrices transposed to [D=128, N=8192], with
    columns rolled per core so each core's own 1024 anchor columns come
    first — the SPMD program is identical on all cores.

Device side (per core, identical program):
  - DMA zpT/zqT into SBUF as float32r (full-rate fp32 TensorEngine mode).
  - For each own 128-row chunk m: S-block = zpT[:, m-block].T @ Z*T against
    all 8192 columns (16 matmuls of [128,512] per matrix into PSUM).
  - ScalarE: exp(2*S) with accum_out => per-row partial sums, 2048 columns
    per ACTIVATE (4 PSUM banks). Only row sums leave the engine.
  - DMA out [128, 64] partial sums (8 m-chunks x 8 groups).

Host finish: neg_i = sum(partials_i) - e^2 (removes the pp diagonal,
exp(2*cos(x,x)) = e^2);  loss_i = log(neg_i) - 2*s_i;  mean over rows.
"""

import numpy as np

N = 8192
D = 128
P = 128
NCORES = 8
M_LOCAL = N // NCORES          # 1024 own rows per core
T_OWN = M_LOCAL // P           # 8 own row chunks
NB = 512                       # matmul moving free dim (one PSUM bank of f32)
GRP = 2048                     # columns per ACT exp instruction (4 banks)
N_GRP = N // GRP               # 4 groups per matrix per row-chunk
OUT_COLS = T_OWN * 2 * N_GRP   # 64 accum columns

EPS = 1e-8
TEMP = 0.5

_CACHE = {}


def _build_nc():
    import concourse.mybir as mybir
    from concourse import bacc
    from concourse.tile import TileContext
    from contextlib import ExitStack

    f32 = mybir.dt.float32
    f32r = mybir.dt.float32r
    AF = mybir.ActivationFunctionType

    nc = bacc.Bacc()
    zpt_d = nc.dram_tensor("zpt", [P, N], f32r, kind="ExternalInput")
    zqt_d = nc.dram_tensor("zqt", [P, N], f32r, kind="ExternalInput")
    out_d = nc.dram_tensor("out", [P, OUT_COLS], f32, kind="ExternalOutput")

    with TileContext(nc) as tc:
        with ExitStack() as ctx:
            sbuf = ctx.enter_context(tc.tile_pool(name="sbuf", bufs=1))
            zpT = sbuf.tile([P, N], f32r)
            zqT = sbuf.tile([P, N], f32r)
            outt = sbuf.tile([P, OUT_COLS], f32)

            # chunked loads so the first matmuls start after ~1 MiB lands
            for g in range(N_GRP):
                cs = slice(g * GRP, (g + 1) * GRP)
                nc.sync.dma_start(out=zpT[:, cs], in_=zpt_d[:, cs])
            for g in range(N_GRP):
                cs = slice(g * GRP, (g + 1) * GRP)
                nc.sync.dma_start(out=zqT[:, cs], in_=zqt_d[:, cs])

            ps_pool = ctx.enter_context(
                tc.tile_pool(name="ps_pool", bufs=2, space="PSUM"))
            scr_pool = ctx.enter_context(tc.tile_pool(name="scr_pool", bufs=2))
            # all pp row-chunks first (needs only zpt), then all pq — the
            # zqt DMA has the whole pp phase (~60us of ACT work) to land
            for mi, zT in enumerate((zpT, zqT)):
                for m in range(T_OWN):
                    lhsT = zpT[:, m * P:(m + 1) * P]
                    for g in range(N_GRP):
                        pt = ps_pool.tile([P, GRP], f32, tag="ps")
                        for s in range(GRP // NB):
                            col = g * GRP + s * NB
                            nc.tensor.matmul(
                                pt[:, s * NB:(s + 1) * NB],
                                lhsT=lhsT,
                                rhs=zT[:, col:col + NB],
                                start=True, stop=True,
                            )
                        scr = scr_pool.tile([P, GRP], f32, tag="scr")
                        acc_col = m * 8 + mi * N_GRP + g
                        nc.scalar.activation(
                            scr[:, :], pt[:, :], AF.Exp, scale=2.0,
                            accum_out=outt[:, acc_col:acc_col + 1],
                        )

            nc.sync.dma_start(out=out_d[:, :], in_=outt[:, :])

    nc.finalize()
    return nc


def _get_nc():
    if "nc" not in _CACHE:
        _CACHE["nc"] = _build_nc()
    return _CACHE["nc"]


def _host_prep(pred, positive):
    """Normalize rows, compute positive-pair logits, build transposed
    per-core (column-rolled) input matrices."""
    def nrm(x):
        n = np.sqrt(np.sum(x * x, axis=1, keepdims=True))
        return x / np.maximum(n, np.float32(EPS))

    zp = nrm(pred)
    zq = nrm(positive)
    s = np.sum(zp.astype(np.float64) * zq.astype(np.float64), axis=1)
    zpT = np.ascontiguousarray(zp.T)   # [D, N]
    zqT = np.ascontiguousarray(zq.T)
    return zpT, zqT, s


LAST_RESULTS = None


def kernel(pred: np.ndarray, positive: np.ndarray) -> np.ndarray:
    global LAST_RESULTS
    import sys
    if "/opt/trn_rl_repo" not in sys.path:
        sys.path.insert(0, "/opt/trn_rl_repo")
    from concourse.bass_utils import run_bass_kernel_spmd

    pred = np.ascontiguousarray(np.asarray(pred, dtype=np.float32))
    positive = np.ascontiguousarray(np.asarray(positive, dtype=np.float32))

    zpT, zqT, s = _host_prep(pred, positive)

    nc = _get_nc()
    in_maps = []
    for c in range(NCORES):
        k = c * M_LOCAL
        in_maps.append({
            "zpt": np.concatenate([zpT[:, k:], zpT[:, :k]], axis=1),
            "zqt": np.concatenate([zqT[:, k:], zqT[:, :k]], axis=1),
        })
    res = run_bass_kernel_spmd(nc, in_maps, core_ids=list(range(NCORES)))
    LAST_RESULTS = res

    # ---- unshard: combine per-core [128, 64] row-sum partials ----
    e2 = np.exp(np.float64(2.0))
    loss_sum = np.float64(0.0)
    for c in range(NCORES):
        o = np.asarray(res.results[c]["out"], dtype=np.float64)
        rowsum = o.reshape(P, T_OWN, 8).sum(axis=2)          # [p, m]
        neg = rowsum - e2
        # row (p, m) of core c is global row c*1024 + m*128 + p
        rows = (c * M_LOCAL
                + np.arange(T_OWN)[None, :] * P
                + np.arange(P)[:, None])
        loss_sum += np.sum(np.log(neg) - 2.0 * s[rows])
    return np.float32(loss_sum / N)



# revision 2
# speedup vs baseline: 1.8377x; 1.8377x over previous
"""SimCLR contrastive loss on 8 TRN2 NeuronCores — v2.

Row-shard the N=8192 anchors across 8 cores (1024 each). Per core the
O(N^2) work is two 1024x8192 similarity blocks (anchor-anchor "pp" and
anchor-positive "pq"), each needing exp(2*s) row sums. v2 improvements
over the 145us baseline:

1) fp8e4 DoubleRow matmuls (0.5 cyc/row, 2x over f32r): inputs are
   host-normalized rows scaled by 16 and quantized to fp8e4m3, laid out
   [64, 2, N] (contraction = 64 partitions x 2 k-tiles).
2) The exp+row-sum bottleneck (ScalarE-only in the baseline) is split
   across THREE engines:
   - ACT: native exp with accum_out (exact path),
   - DVE: Schraudolph exp — tensor_scalar converts PSUM f32 s-values to
     int16 bf16-bit-codes (round-to-nearest, verified on HW), a second
     4x-mode tensor_scalar sums the bitcast bf16 values,
   - Pool: partition_all_reduce column sums of the Schraudolph codes.
3) pp symmetry: each core computes only 5 of 8 column blocks of its pp
   row-block (rolled cols [0, 5*1024)). Column sums of blocks 1..3
   (on Pool) provide the missing d in {5,6,7} partner contributions:
   for anchor i in core a, row sums cover j in cores a..a+4 and partner
   col sums cover j in cores a+5..a+7 — exact, no double count.

Host finish: combine ACT partials (exact), DVE/Pool partials (/KS
Schraudolph calibration), subtract the pp diagonal (recomputed exactly
from the fp8 values), loss_i = log(neg_i) - 2*s_i, mean.
"""

import numpy as np

N = 8192
D = 128
P = 128
NCORES = 8
M_LOCAL = N // NCORES          # 1024 own rows per core
T_OWN = M_LOCAL // P           # 8 own row chunks
TILE = 1024                    # consumer tile (cols); 2 PSUM banks f32
PP_BLOCKS = 5                  # pp col blocks computed (of 8)
SYM_T = (1, 2, 3)              # pp blocks whose col sums feed partners

EPS = 1e-8
KAPPA = 16.0                   # fp8 pre-scale; PSUM s' = 256*s
ACT_SCALE = 2.0 / (KAPPA * KAPPA)   # exp(2s) from PSUM value
# Schraudolph (bf16 codes, RNE convert — verified on HW):
#   i16 = rne(A1*psum + B1); bf16bits(i16) ~ exp(2s) * KS
A1 = float(np.float32(2.0 * (128.0 / np.log(2.0)) / 256.0))
B1 = 16250.0                   # 127*128 - 6
KS = 1.000910                  # E[schraudolph/exp], calibrated for B1

_CACHE = {}


def _schedule():
    """Static tile schedule: list of (mat, m, t, stream, sym).
    stream 'A' = ACT exp path, 'D' = DVE Schraudolph path.
    Greedy engine balance using the TimelineSim per-instr cost model."""
    act_c = 0.8333 * TILE + 362.0
    dve_c = (1.0417 * TILE + 170.0) + (0.26 * TILE + 105.0)
    clocks = {"A": 0.0, "D": 0.0}
    sched = []
    for m in range(T_OWN):
        for mat in (0, 1):
            nb = PP_BLOCKS if mat == 0 else 8
            for t in range(nb):
                sym = (mat == 0 and t in SYM_T)
                if sym:
                    st = "D"
                elif mat == 0 and t == 0:
                    st = "A"   # contains the pp diagonal — keep exact
                else:
                    st = ("A" if clocks["A"] + act_c <= clocks["D"] + dve_c
                          else "D")
                clocks[st] += act_c if st == "A" else dve_c
                sched.append((mat, m, t, st, sym))
    return sched


SCHED = _schedule()
N_ACT = sum(1 for e in SCHED if e[3] == "A")
N_DVE = sum(1 for e in SCHED if e[3] == "D")
N_SYM = sum(1 for e in SCHED if e[4])


def _build_nc():
    import concourse.mybir as mybir
    import concourse.bass_isa as bass_isa
    from concourse import bacc
    from concourse.tile import TileContext
    from contextlib import ExitStack

    f32 = mybir.dt.float32
    bf16 = mybir.dt.bfloat16
    i16 = mybir.dt.int16
    fp8 = mybir.dt.float8e4
    AF = mybir.ActivationFunctionType
    ALU = mybir.AluOpType
    DR = mybir.MatmulPerfMode.DoubleRow

    nc = bacc.Bacc()
    zp_d = nc.dram_tensor("zpt", [64, 2, N], fp8, kind="ExternalInput")
    zq_d = nc.dram_tensor("zqt", [64, 2, N], fp8, kind="ExternalInput")
    outa_d = nc.dram_tensor("outa", [P, max(N_ACT, 1)], f32,
                            kind="ExternalOutput")
    outd_d = nc.dram_tensor("outd", [P, max(N_DVE, 1)], f32,
                            kind="ExternalOutput")
    cs_d = nc.dram_tensor("cs", [N_SYM, TILE], f32, kind="ExternalOutput")

    with TileContext(nc) as tc:
        with ExitStack() as ctx:
            sbuf = ctx.enter_context(tc.tile_pool(name="sbuf", bufs=1))
            z3p = sbuf.tile([64, 2, N], fp8)
            z3q = sbuf.tile([64, 2, N], fp8)
            outa = sbuf.tile([P, max(N_ACT, 1)], f32)
            outd = sbuf.tile([P, max(N_DVE, 1)], f32)
            trash = sbuf.tile([P, TILE], bf16)

            # chunked input loads so compute can start early
            NCH = 4
            CW = N // NCH
            for g in range(NCH):
                cs = slice(g * CW, (g + 1) * CW)
                nc.sync.dma_start(out=z3p[:, :, cs], in_=zp_d[:, :, cs])
            for g in range(NCH):
                cs = slice(g * CW, (g + 1) * CW)
                nc.sync.dma_start(out=z3q[:, :, cs], in_=zq_d[:, :, cs])

            act_ps = ctx.enter_context(
                tc.tile_pool(name="act_ps", bufs=2, space="PSUM"))
            dve_ps = ctx.enter_context(
                tc.tile_pool(name="dve_ps", bufs=2, space="PSUM"))
            q_pool = ctx.enter_context(tc.tile_pool(name="q_pool", bufs=4))
            cs_pool = ctx.enter_context(tc.tile_pool(name="cs_pool", bufs=2))

            ia = idv = isym = 0
            for (mat, m, t, st, sym) in SCHED:
                zr = z3p if mat == 0 else z3q
                lhsT = z3p[:, :, m * P:(m + 1) * P]
                pool = act_ps if st == "A" else dve_ps
                pt = pool.tile([P, TILE], f32, tag="a" if st == "A" else "d")
                for j in range(TILE // 256):
                    c0 = t * TILE + j * 256
                    nc.tensor.matmul(
                        pt[:, j * 256:(j + 1) * 256],
                        lhsT=lhsT, rhs=zr[:, :, c0:c0 + 256],
                        start=True, stop=True, perf_mode=DR)
                if st == "A":
                    nc.scalar.activation(
                        pt[:, :], pt[:, :], AF.Exp, scale=ACT_SCALE,
                        accum_out=outa[:, ia:ia + 1])
                    ia += 1
                else:
                    qt = q_pool.tile([P, TILE], i16, tag="q")
                    nc.vector.tensor_scalar(qt[:, :], pt[:, :], A1, B1,
                                            ALU.mult, ALU.add)
                    nc.vector.tensor_scalar(
                        trash[:, :], qt[:, :].bitcast(bf16), 1.0, 0.0,
                        ALU.mult, ALU.add, accum_out=outd[:, idv:idv + 1])
                    idv += 1
                    if sym:
                        cst = cs_pool.tile([P, TILE], f32, tag="cs")
                        nc.gpsimd.partition_all_reduce(
                            cst[:, :], qt[:, :].bitcast(bf16), 128,
                            bass_isa.ReduceOp.add)
                        nc.sync.dma_start(out=cs_d[isym:isym + 1, :],
                                          in_=cst[0:1, :])
                        isym += 1

            nc.sync.dma_start(out=outa_d[:, :], in_=outa[:, :])
            nc.sync.dma_start(out=outd_d[:, :], in_=outd[:, :])

    nc.finalize()
    return nc


def _get_nc():
    if "nc" not in _CACHE:
        _CACHE["nc"] = _build_nc()
    return _CACHE["nc"]


def _host_prep(pred, positive):
    import ml_dtypes

    def nrm(x):
        n = np.sqrt(np.sum(x * x, axis=1, keepdims=True))
        return x / np.maximum(n, np.float32(EPS))

    zp = nrm(pred)
    zq = nrm(positive)
    s = np.sum(zp.astype(np.float64) * zq.astype(np.float64), axis=1)
    zp8 = (zp.T * np.float32(KAPPA)).astype(ml_dtypes.float8_e4m3)  # [D, N]
    zq8 = (zq.T * np.float32(KAPPA)).astype(ml_dtypes.float8_e4m3)
    # device-exact pp diagonal: sum_d fp8(16 zp)^2 / 256, exp(2s_ii~)
    dd = np.sum(zp8.astype(np.float64) ** 2, axis=0)
    diag_exp = np.exp(dd * ACT_SCALE)
    return zp8, zq8, s, diag_exp


LAST_RESULTS = None


def kernel(pred: np.ndarray, positive: np.ndarray) -> np.ndarray:
    global LAST_RESULTS
    import sys
    if "/opt/trn_rl_repo" not in sys.path:
        sys.path.insert(0, "/opt/trn_rl_repo")
    from concourse.bass_utils import run_bass_kernel_spmd

    pred = np.ascontiguousarray(np.asarray(pred, dtype=np.float32))
    positive = np.ascontiguousarray(np.asarray(positive, dtype=np.float32))

    zp8, zq8, s, diag_exp = _host_prep(pred, positive)

    def roll3(z8, k):
        r = np.concatenate([z8[:, k:], z8[:, :k]], axis=1)      # [128, N]
        return np.ascontiguousarray(r.reshape(2, 64, N).transpose(1, 0, 2))

    nc = _get_nc()
    in_maps = []
    for c in range(NCORES):
        k = c * M_LOCAL
        in_maps.append({"zpt": roll3(zp8, k), "zqt": roll3(zq8, k)})
    res = run_bass_kernel_spmd(nc, in_maps, core_ids=list(range(NCORES)))
    LAST_RESULTS = res

    neg = np.zeros(N, dtype=np.float64)
    inv_ks = 1.0 / KS
    for c in range(NCORES):
        oa = np.asarray(res.results[c]["outa"], dtype=np.float64)
        od = np.asarray(res.results[c]["outd"], dtype=np.float64)
        cs = np.asarray(res.results[c]["cs"], dtype=np.float64)
        ia = idv = isym = 0
        for (mat, m, t, st, sym) in SCHED:
            rows = c * M_LOCAL + m * P + np.arange(P)
            if st == "A":
                neg[rows] += oa[:, ia]
                ia += 1
            else:
                neg[rows] += od[:, idv] * inv_ks
                idv += 1
                if sym:
                    anchors = (c * M_LOCAL + t * TILE + np.arange(TILE)) % N
                    neg[anchors] += cs[isym, :] * inv_ks
                    isym += 1
    neg -= diag_exp
    loss = np.mean(np.log(neg) - 2.0 * s)
    return np.float32(loss)
